# revision 1
# baseline (speedup 1.0000x reference)
"""
AM-Softmax + intra-class loss kernel for Trainium2, 8 NeuronCores.

Strategy (class-sharded distributed softmax):
  * Classes C=20000 are sharded 2500/core (padded to 2560 = 5 x 512 tiles).
    Every core holds the full embedding matrix E [4096, 256].
  * Per core: Z = E @ (30 * W_norm_shard).T via f32r matmuls; the per-row
    1/||E_i|| is the ACT per-partition scale of the exp, so E is never
    materialised normalised and the AM scale rides on W.
  * No row-max pass: cos <= 1 so s*cos <= 30 is a valid logsumexp offset.
    Each core returns S_i = sum_shard exp(s*cos - 30) (exact logsumexp math).
  * exp+row-sum fused on ACT reading PSUM directly; 2 wide activations per
    row chunk (1536 + 964 cols) amortise the 352-cycle ACT op overhead; pad
    columns are never exp'd; the B-half row-sum runs on DVE to skip the
    fixed ACT accumulator drain. All 1/||row|| factors use a DVE-only
    Newton rsqrt (magic-constant seed), so ACT executes ONLY Square+Exp --
    one LUT table set, loaded once. E.T is host-pre-transposed (layout
    move); W is normalised (x30) and PE-transposed on device, chunks 12-19
    prepped mid-loop so the A-phase never waits on the last W transfer.
  * Label logits: host gathers W[labels] rows (data movement only), device
    computes the row-wise dot + both norms -> cos at the label, 512 rows/core.
  * Intra-class term: for group g, sum_{i<j} (1 - e_i.e_j) =
    28 - (||sum_g e||^2 - 8)/2, so one selection-matmul + square-accumulate
    gives all 64 groups of a core. Host combines (O(B) work, float64).
"""

import numpy as np

import concourse.bacc as bacc
import concourse.bass as bass
import concourse.tile as tile
from concourse import mybir
from concourse.bass_utils import run_bass_kernel_spmd
from concourse.masks import make_identity

B = 4096
D = 256
C = 20000
G = 512
NSAMP = 8           # samples per group
NCORES = 8
CREAL = C // NCORES          # 2500 real classes per core
NTILE = 512                  # matmul moving free dim / PSUM bank
NT = 5                       # 512-wide matmul tiles per core
CSH = NT * NTILE             # 2560 padded classes per core
RCH = B // 128               # 32 row chunks
WCH = CSH // 128             # 20 w chunks
RPC = B // NCORES            # 512 rows per core (for label cos)
GPC = G // NCORES            # 64 groups per core
CA = 3 * NTILE               # first exp chunk: 1536 cols
CB = CREAL - CA              # second exp chunk: 964 real cols (of 1024)

AM_MARGIN = 0.3
AM_SCALE = 30.0
INTRA_MARGIN = 0.5
LAMBDA_INTRA = 0.1
OFF = 30.0                   # fixed logsumexp offset (= AM_SCALE * max cos)

F32 = mybir.dt.float32
F32R = mybir.dt.float32r
I32 = mybir.dt.int32
AF = mybir.ActivationFunctionType
ALU = mybir.AluOpType
AXL = mybir.AxisListType


def build_program():
    nc = bacc.Bacc("TRN2", target_bir_lowering=False)

    e_d = nc.dram_tensor("e", [B, D], F32, kind="ExternalInput")
    et_d = nc.dram_tensor("et", [D, B], F32R, kind="ExternalInput")
    w_d = nc.dram_tensor("w", [CSH, D], F32, kind="ExternalInput")
    er_d = nc.dram_tensor("er", [RPC, D], F32, kind="ExternalInput")
    wl_d = nc.dram_tensor("wl", [RPC, D], F32, kind="ExternalInput")
    eg_d = nc.dram_tensor("eg", [RPC, D], F32, kind="ExternalInput")
    sel_d = nc.dram_tensor("sel", [128, GPC], F32, kind="ExternalInput")

    out_s = nc.dram_tensor("out_s", [128, RCH], F32, kind="ExternalOutput")
    out_lc = nc.dram_tensor("out_lc", [128, 12], F32, kind="ExternalOutput")
    out_iv = nc.dram_tensor("out_iv", [GPC, 1], F32, kind="ExternalOutput")

    from contextlib import ExitStack
    with tile.TileContext(nc) as tc, ExitStack() as ctx:
        big = ctx.enter_context(tc.tile_pool(name="big", bufs=1))
        scr = ctx.enter_context(tc.tile_pool(name="scr", bufs=3))
        psum = ctx.enter_context(tc.tile_pool(name="psum", bufs=2, space="PSUM"))
        tpsum = ctx.enter_context(tc.tile_pool(name="tpsum", bufs=2, space="PSUM"))

        ident = big.tile([128, 128], F32)
        make_identity(nc, ident)

        def sumsq4(src4, dst4):
            """dst4[128,4] = row sum-of-squares of 4 chunks [128,4,256], DVE."""
            s = scr.tile([128, 4, D], F32, tag="sq4")
            nc.vector.tensor_mul(s, src4, src4)
            nc.vector.tensor_reduce(out=dst4, in_=s, axis=AXL.X, op=ALU.add)

        NWT = 16  # all rsqrt batches padded to one width so scratch slots share

        def rsqrt_dve(dst, x, n, scale=1.0):
            """dst[:, :n] = scale/sqrt(x[:, :n]) on DVE only (magic-constant
            seed + 3 Newton steps). Keeps sqrt off ACT so the whole kernel
            stays in the exp_and_others LUT set."""
            yi = scr.tile([128, NWT], I32, tag="nwty")
            nc.vector.tensor_scalar(out=yi[:, :n], in0=x.bitcast(I32),
                                    scalar1=1, scalar2=None,
                                    op0=ALU.arith_shift_right)
            # 0x5f3759df - s  ==  (~s) + 0x5f3759e0
            nc.vector.tensor_scalar(out=yi[:, :n], in0=yi[:, :n],
                                    scalar1=-1, scalar2=None,
                                    op0=ALU.bitwise_xor)
            nc.vector.tensor_scalar(out=yi[:, :n], in0=yi[:, :n],
                                    scalar1=0x5f3759e0, scalar2=None,
                                    op0=ALU.add)
            y = yi.bitcast(F32)
            t = scr.tile([128, NWT], F32, tag="nwtt")
            for it in range(3):
                nc.vector.tensor_mul(t[:, :n], y[:, :n], y[:, :n])
                nc.vector.tensor_mul(t[:, :n], t[:, :n], x)
                last = it == 2
                nc.vector.tensor_scalar(
                    out=t[:, :n], in0=t[:, :n],
                    scalar1=(-0.5 * scale) if last else -0.5,
                    scalar2=(1.5 * scale) if last else 1.5,
                    op0=ALU.mult, op1=ALU.add)
                nc.vector.tensor_mul(dst if last else y[:, :n], y[:, :n],
                                     t[:, :n])

        # ---------------- input DMAs, critical-path order --------------------
        # SWDGE (gpsimd) queue: the small tensors; eg first (gates an ACT sqrt)
        egsb = big.tile([128, RPC // 128, D], F32)
        selsb = big.tile([128, GPC], F32)
        ersb = big.tile([128, RPC // 128, D], F32)
        wlsb = big.tile([128, RPC // 128, D], F32)
        # SP queue: W (3 transfers, so norm work can stream), then E
        wsb = big.tile([128, WCH, D], F32)
        esb = big.tile([128, RCH, D], F32)
        ET = big.tile([128, 2, B], F32R)

        def et_dma(q):
            nc.sync.dma_start(
                out=ET[:, :, q * 1024:(q + 1) * 1024],
                in_=et_d[:].rearrange("(kd p) r -> p kd r", p=128)[:, :, q * 1024:(q + 1) * 1024])

        # order: W first (gates the whole left path), E natural (row norms
        # gate every exp), then E.T quarters (feed the matmuls).
        def e_dma(h):
            nc.sync.dma_start(
                out=esb[:, h * 8:(h + 1) * 8],
                in_=e_d[:].rearrange("(c p) d -> p c d", p=128)[:, h * 8:(h + 1) * 8])

        def w_dma(a, b):
            nc.sync.dma_start(
                out=wsb[:, a:b],
                in_=w_d[:].rearrange("(c p) d -> p c d", p=128)[:, a:b])

        e_dma(0)
        w_dma(0, 8)        # A-phase needs only chunks 0-11...
        e_dma(1)
        w_dma(8, 12)       # ...split around e so the square stream never stalls
        e_dma(2)
        e_dma(3)
        et_dma(0)
        w_dma(12, 20)      # chunks 12-19: prepped mid-loop, B-phase is late
        nc.sync.dma_start(out=egsb, in_=eg_d[:].rearrange("(c p) d -> p c d", p=128))
        nc.sync.dma_start(out=selsb, in_=sel_d[:])
        for q in range(1, 4):
            et_dma(q)
        nc.sync.dma_start(out=ersb, in_=er_d[:].rearrange("(c p) d -> p c d", p=128))
        nc.sync.dma_start(out=wlsb, in_=wl_d[:].rearrange("(c p) d -> p c d", p=128))

        # ---------------- norm factors (all ACT sqrts happen here) -----------
        wsq = big.tile([128, WCH], F32)
        winv = big.tile([128, WCH], F32)
        for g in range(3):
            sl = slice(4 * g, 4 * g + 4)
            sumsq4(wsb[:, sl], wsq[:, sl])
        rsqrt_dve(winv[:, 0:12], wsq[:, 0:12], 12, scale=float(AM_SCALE))

        # ---------------- W scale + transpose, E transpose -------------------
        WT = big.tile([128, 2, CSH], F32R)

        def w_prep(c):
            nc.vector.tensor_scalar_mul(wsb[:, c], wsb[:, c], winv[:, c:c + 1])
            pt = tpsum.tile([128, 2, 128], F32, tag="tp")
            for kd in range(2):
                nc.tensor.transpose(pt[:, kd], wsb[:, c, kd * 128:(kd + 1) * 128],
                                    ident)
            nc.vector.tensor_copy(out=WT[:, :, c * 128:(c + 1) * 128], in_=pt)

        # A-phase needs only chunks 0-11; the rest are prepared during the
        # A-phase so PE's in-order stream never stalls on the last W DMA.
        for c in range(12):
            w_prep(c)
        # E row sumsq on ACT (Square shares the sqrt LUT set); 1/||E|| via
        # DVE recip + ACT sqrt, in two halves so rows 0-15 exp early.
        esq = big.tile([128, RCH], F32)
        sinv = big.tile([128, RCH], F32)
        for c in range(RCH):
            sq = scr.tile([128, D], F32, tag="sqact")
            nc.scalar.activation(out=sq, in_=esb[:, c], func=AF.Square,
                                 accum_out=esq[:, c:c + 1])
        for h in range(2):
            sl = slice(h * 16, h * 16 + 16)
            rsqrt_dve(sinv[:, sl], esq[:, sl], 16)

        # ACT now runs only Square/Exp (one LUT set) -> no ordering gate.
        negoff = big.tile([128, 1], F32)
        nc.vector.memset(negoff, -OFF)

        # ---------------- main loop: Z tiles -> exp-accumulate ---------------
        # A-phase: first 1536 cols for every row chunk; B-phase: the rest.
        # Two PSUM macro-tiles in flight; one wide exp+accum per macro-tile.
        tsums = big.tile([128, RCH, 2], F32)
        for half in range(2):
            if half == 1:
                for g in range(3, 5):
                    sl = slice(4 * g, 4 * g + 4)
                    sumsq4(wsb[:, sl], wsq[:, sl])
                rsqrt_dve(winv[:, 12:20], wsq[:, 12:20], 8,
                          scale=float(AM_SCALE))
                for c in range(12, WCH):
                    w_prep(c)
            c0, ncols = ((0, CA), (CA, CB))[half]
            nbanks = (CSH - CA) // NTILE if half else CA // NTILE
            for r in range(RCH):
                pt = psum.tile([128, CA], F32, tag="mm")
                for tb in range(nbanks):
                    for kd in range(2):
                        nc.tensor.matmul(
                            pt[:, tb * NTILE:(tb + 1) * NTILE],
                            lhsT=ET[:, kd, r * 128:(r + 1) * 128],
                            rhs=WT[:, kd, c0 + tb * NTILE:c0 + (tb + 1) * NTILE],
                            start=(kd == 0), stop=(kd == 1))
                s1 = scr.tile([128, CA], F32, tag="expscr")
                if half == 0:
                    nc.scalar.activation(
                        out=s1[:, :ncols], in_=pt[:, :ncols], func=AF.Exp,
                        scale=sinv[:, r:r + 1], bias=negoff[:, 0:1],
                        accum_out=tsums[:, r, half:half + 1])
                else:
                    # B row-sum on DVE: saves the fixed ACT accumulator drain
                    nc.scalar.activation(
                        out=s1[:, :ncols], in_=pt[:, :ncols], func=AF.Exp,
                        scale=sinv[:, r:r + 1], bias=negoff[:, 0:1])
                    nc.vector.tensor_reduce(out=tsums[:, r, 1:2],
                                            in_=s1[:, :ncols],
                                            axis=AXL.X, op=ALU.add)

        sums = big.tile([128, RCH], F32)
        nc.vector.tensor_reduce(out=sums, in_=tsums, axis=AXL.X, op=ALU.add)
        nc.sync.dma_start(out=out_s[:], in_=sums)

        # ---------------- tail: intra + label-cos raw pieces ------------------
        egsq = big.tile([128, RPC // 128], F32)
        eginv = big.tile([128, RPC // 128], F32)
        sumsq4(egsb, egsq)
        rsqrt_dve(eginv, egsq, RPC // 128)
        for j in range(RPC // 128):
            nc.vector.tensor_scalar_mul(egsb[:, j], egsb[:, j], eginv[:, j:j + 1])
        sg = tpsum.tile([GPC, D], F32, tag="tp")
        for j in range(RPC // 128):
            nc.tensor.matmul(sg, lhsT=selsb, rhs=egsb[:, j],
                             start=(j == 0), stop=(j == RPC // 128 - 1))
        ssq = big.tile([GPC, 1], F32)
        sgsb = scr.tile([GPC, D], F32, tag="sgsb")
        nc.vector.tensor_copy(sgsb, sg)
        sgscr = scr.tile([GPC, D], F32, tag="sgscr")
        nc.vector.tensor_mul(sgscr, sgsb, sgsb)
        nc.vector.tensor_reduce(out=ssq, in_=sgscr, axis=AXL.X, op=ALU.add)
        # per_group = relu(mean_d - margin), mean_d = 1 - (ssq - n)/(2*npairs)
        npairs = NSAMP * (NSAMP - 1) / 2.0
        iv = big.tile([GPC, 1], F32)
        nc.vector.tensor_scalar(out=iv, in0=ssq,
                                scalar1=-1.0 / (2.0 * npairs),
                                scalar2=(1.0 - INTRA_MARGIN) + NSAMP / (2.0 * npairs),
                                op0=ALU.mult, op1=ALU.add)
        nc.vector.tensor_scalar_max(iv, iv, 0.0)
        nc.sync.dma_start(out=out_iv[:], in_=iv)

        # lcpack: cols 0:4 = <er,wl>, 4:8 = sumsq(er), 8:12 = sumsq(wl).
        # Host does lc = tt / sqrt(ersq*wlsq) -- keeps sqrts off ACT here.
        lcpack = big.tile([128, 12], F32)
        sumsq4(ersb, lcpack[:, 4:8])
        sumsq4(wlsb, lcpack[:, 8:12])
        for j in range(RPC // 128):
            s1 = scr.tile([128, D], F32, tag="ttscr")
            nc.vector.tensor_mul(s1, ersb[:, j], wlsb[:, j])
            nc.vector.tensor_reduce(out=lcpack[:, j:j + 1], in_=s1,
                                    axis=AXL.X, op=ALU.add)
        nc.sync.dma_start(out=out_lc[:], in_=lcpack)

    nc.finalize()
    return nc


def kernel(embeddings, labels, weight):
    e = np.ascontiguousarray(embeddings, dtype=np.float32)
    lab = np.asarray(labels).astype(np.int64)
    w = np.ascontiguousarray(weight, dtype=np.float32)
    assert e.shape == (B, D) and w.shape == (C, D) and lab.shape == (B,)

    # group membership (derived from labels; fill is arange % G)
    members = np.argsort(lab, kind="stable").reshape(G, NSAMP)  # [G, 8] row idx
    assert np.all(lab[members[:, 0]] == np.arange(G))

    sel = np.tile(np.eye(GPC, dtype=np.float32), (2, 1))  # [128, 64]
    et = np.ascontiguousarray(e.T)                        # [D, B] layout move

    in_maps = []
    for k in range(NCORES):
        wsh = np.empty((CSH, D), np.float32)
        wsh[:CREAL] = w[k * CREAL:(k + 1) * CREAL]
        wsh[CREAL:] = 1.0
        rows = slice(k * RPC, (k + 1) * RPC)
        er = e[rows]
        wl = np.ascontiguousarray(w[lab[rows]])
        # intra rows for groups [64k, 64k+64), ordered sample-major (j, t)
        gm = members[k * GPC:(k + 1) * GPC]          # [64, 8]
        eg_idx = gm.T.reshape(-1)                    # j-major: row j*64+t
        eg = np.ascontiguousarray(e[eg_idx])
        in_maps.append({
            "e": e, "et": et, "w": wsh,
            "er": np.ascontiguousarray(er), "wl": wl,
            "eg": eg, "sel": sel,
        })

    nc = build_program()
    res = run_bass_kernel_spmd(nc, in_maps, core_ids=list(range(NCORES)))
    global _last_results
    _last_results = res

    # ---------------- host combine (O(B), float64) -----------------------
    S = np.zeros(B, np.float64)
    for k in range(NCORES):
        S += res.results[k]["out_s"].T.reshape(B).astype(np.float64)
    cls = []
    for k in range(NCORES):
        pk = res.results[k]["out_lc"].astype(np.float64)
        tt = pk[:, 0:4].T.reshape(RPC)
        ersq = pk[:, 4:8].T.reshape(RPC)
        wlsq = pk[:, 8:12].T.reshape(RPC)
        cls.append(tt / np.sqrt(ersq * wlsq))
    cl = np.concatenate(cls)

    s, m = float(AM_SCALE), float(AM_MARGIN)
    S_adj = S - np.exp(s * cl - OFF) + np.exp(s * (cl - m) - OFF)
    am_i = (np.log(S_adj) + OFF) - s * (cl - m)
    am = am_i.mean()

    ivals = np.concatenate(
        [res.results[k]["out_iv"][:, 0] for k in range(NCORES)]
    ).astype(np.float64)
    intra = ivals.sum() / G
    total = am + LAMBDA_INTRA * intra
    return (np.float32(total), np.float32(am), np.float32(intra))



# revision 15
# speedup vs baseline: 1.1951x; 1.1951x over previous
"""
AM-Softmax + intra-class loss kernel for Trainium2, 8 NeuronCores.

Strategy (class-sharded distributed softmax, fp8 matmuls, 3-engine exp):
  * Classes C=20000 sharded 2500/core (padded 2560 = 20 x 128 chunks).
    Matmuls run in fp8e4m3 with DoubleRow perf mode (contract 256 in one
    pass, 0.5 cyc/row): E.T and W arrive as host-cast fp8 (raw values; the
    normalisations are folded in on device, so casting is pure data
    movement).
  * Row/class norms: ||q8(e)||^2 and ||q8(w)||^2 are computed in the
    "flipped" layout - elementwise squares on DVE (2x mode) followed by
    PE ones-matmuls that contract the partition axis - then bounced
    DRAM->SBUF to land as per-partition scalars.  No ACT Square ops at
    all: ACT runs ONLY Exp (one LUT set).
  * Main loop per 128-row chunk: psum A [128,1536] (chunks 0-11) is
    exp'd by ACT (scale=1/||e||, bias=-30, fused row-sum accumulator);
    psum B 2x[128,512] (chunks 12-19) is drained by DVE+Pool with a
    Schraudolph fast-exp (t = z*(rinv*c1) + c2 -> int16, bitcast bf16),
    then summed by a DVE bf16 pairwise tree + short reduce.  Pad classes
    get zero int16 codes (= +0.0 bf16) so they vanish from the sums.
  * cos <= 1 so s*cos <= 30 is a fixed logsumexp offset (no max pass).
  * Label logits: host gathers W[labels] rows; device computes the
    row-dot and both sumsq in bf16; host finishes cos + adjustment.
  * Intra-class term: sum_{i<j}(1 - e_i.e_j) = 28 - (||sum_g e||^2-8)/2
    via one selection-matmul per core; host combines in float64.
"""

import numpy as np
import ml_dtypes

import concourse.bacc as bacc
import concourse.bass as bass
import concourse.tile as tile
from concourse import mybir
from concourse.bass_utils import run_bass_kernel_spmd
from concourse.masks import make_identity

B = 4096
D = 256
C = 20000
G = 512
NSAMP = 8
NCORES = 8
CREAL = C // NCORES          # 2500 real classes per core
CSH = 2560                   # padded classes per core (20 x 128)
WCH = CSH // 128             # 20 class chunks
RCH = B // 128               # 32 row chunks
RPC = B // NCORES            # 512 rows per core (label path)
GPC = G // NCORES            # 64 groups per core
CA = 1536                    # ACT share (chunks 0-11)
NB = 2                       # B psum tiles of 512 (chunks 12-19)

AM_MARGIN = 0.3
AM_SCALE = 30.0
INTRA_MARGIN = 0.5
LAMBDA_INTRA = 0.1
OFF = 30.0

# Schraudolph fast-exp constants (bf16 target: 8 exp bits, 7 mantissa bits)
C1 = 128.0 * 1.4426950408889634          # 184.66496...
BADJ = 7.25                              # tuned to zero the mean ratio error
C2 = 16256.0 - BADJ - OFF * C1           # additive code constant

F32 = mybir.dt.float32
F8 = mybir.dt.float8e4
BF16 = mybir.dt.bfloat16
I16 = mybir.dt.int16
I32 = mybir.dt.int32
AF = mybir.ActivationFunctionType
ALU = mybir.AluOpType
AXL = mybir.AxisListType
DR = mybir.MatmulPerfMode.DoubleRow

# DVE-share columns of the second B tile (rest goes to Pool)
DVE_B2 = 0   # DVE takes B1 fully; Pool takes B2's real 452 cols


def build_program():
    nc = bacc.Bacc("TRN2", target_bir_lowering=False)

    et8_d = nc.dram_tensor("et8", [128, 2, B], F8, kind="ExternalInput")
    w8_d = nc.dram_tensor("w8", [128, WCH, D], F8, kind="ExternalInput")
    wt8_d = nc.dram_tensor("wt8", [128, 2, CSH], F8, kind="ExternalInput")
    er_d = nc.dram_tensor("er", [128, 4, D], BF16, kind="ExternalInput")
    wl_d = nc.dram_tensor("wl", [128, 4, D], BF16, kind="ExternalInput")
    eg_d = nc.dram_tensor("eg", [128, 4, D], BF16, kind="ExternalInput")
    sel_d = nc.dram_tensor("sel", [128, GPC], BF16, kind="ExternalInput")

    esq_scr = nc.dram_tensor("esq_scr", [8, 512], F32, kind="Internal")
    wsq_scr = nc.dram_tensor("wsq_scr", [5, 512], F32, kind="Internal")

    out_s = nc.dram_tensor("out_s", [128, RCH], F32, kind="ExternalOutput")
    out_lc = nc.dram_tensor("out_lc", [128, 12], F32, kind="ExternalOutput")
    out_iv = nc.dram_tensor("out_iv", [GPC, 1], F32, kind="ExternalOutput")

    from contextlib import ExitStack
    with tile.TileContext(nc) as tc, ExitStack() as ctx:
        big = ctx.enter_context(tc.tile_pool(name="big", bufs=1))
        scr = ctx.enter_context(tc.tile_pool(name="scr", bufs=2))

        NWT = 32

        def rsqrt_dve(dst, x, n, scale=1.0):
            """dst[:, :n] = scale/sqrt(x[:, :n]) DVE-only Newton rsqrt."""
            yi = scr.tile([128, NWT], I32, tag="nwty")
            nc.vector.tensor_scalar(out=yi[:, :n], in0=x.bitcast(I32),
                                    scalar1=1, scalar2=None,
                                    op0=ALU.arith_shift_right)
            nc.vector.tensor_scalar(out=yi[:, :n], in0=yi[:, :n],
                                    scalar1=-1, scalar2=None,
                                    op0=ALU.bitwise_xor)
            nc.vector.tensor_scalar(out=yi[:, :n], in0=yi[:, :n],
                                    scalar1=0x5f3759e0, scalar2=None,
                                    op0=ALU.add)
            y = yi.bitcast(F32)
            t = scr.tile([128, NWT], F32, tag="nwtt")
            for it in range(3):
                nc.vector.tensor_mul(t[:, :n], y[:, :n], y[:, :n])
                nc.vector.tensor_mul(t[:, :n], t[:, :n], x)
                last = it == 2
                nc.vector.tensor_scalar(
                    out=t[:, :n], in0=t[:, :n],
                    scalar1=(-0.5 * scale) if last else -0.5,
                    scalar2=(1.5 * scale) if last else 1.5,
                    op0=ALU.mult, op1=ALU.add)
                nc.vector.tensor_mul(dst if last else y[:, :n], y[:, :n],
                                     t[:, :n])

        # ---------------- input DMAs ----------------------------------------
        w8 = big.tile([128, WCH, D], F8)
        wt8 = big.tile([128, 2, CSH], F8)
        et8 = big.tile([128, 2, B], F8)
        ersb = big.tile([128, 4, D], BF16)
        wlsb = big.tile([128, 4, D], BF16)
        egsb = big.tile([128, 4, D], BF16)
        selsb = big.tile([128, GPC], BF16)

        nc.sync.dma_start(out=wt8[:, :, 0:1280], in_=wt8_d[:, :, 0:1280])
        nc.sync.dma_start(out=wt8[:, :, 1280:2560], in_=wt8_d[:, :, 1280:2560])
        nc.sync.dma_start(out=w8, in_=w8_d[:])

        ones_bf = big.tile([128, 1], BF16)
        nc.vector.memset(ones_bf, 1.0)
        identb = big.tile([128, 128], BF16)
        make_identity(nc, identb)
        negoff = big.tile([128, 1], F32)
        nc.vector.memset(negoff, -OFF)
        # force the Exp LUT table load off the critical path
        warmup = big.tile([128, 1], BF16)
        nc.scalar.activation(out=warmup, in_=negoff, func=AF.Exp)

        esq = big.tile([128, RCH], F32)
        wsq = big.tile([128, WCH], F32)
        rinv = big.tile([128, RCH], F32)
        rinvc1 = big.tile([128, RCH], F32)
        winv = big.tile([128, WCH], F32)
        WT8 = big.tile([128, 2, CSH], F8)

        # ---------------- prep phase: norms via flipped ones-matmuls --------
        with tc.tile_pool(name="prep", bufs=1, space="PSUM") as pp, \
             tc.tile_pool(name="tp", bufs=2, space="PSUM") as tp:
            # wsq: wt8 o wt8 -> bf16, then ones-matmul per 512-block
            wsqsrc = big.tile([128, 2, CSH], BF16)
            for kd in range(2):
                for q in range(5):
                    sli = slice(q * 512, (q + 1) * 512)
                    nc.vector.tensor_mul(wsqsrc[:, kd, sli], wt8[:, kd, sli],
                                         wt8[:, kd, sli])
            pw = []
            for i in range(2):
                pwt = pp.tile([128, 512], F32, tag=f"pw{i}")
                pw.append(pwt)
            for j in range(5):
                dst = pw[j // 3][(j % 3) * 32:(j % 3) * 32 + 1, :]
                for kd in range(2):
                    nc.tensor.matmul(dst, lhsT=ones_bf,
                                     rhs=wsqsrc[:, kd, j * 512:(j + 1) * 512],
                                     start=(kd == 0), stop=(kd == 1))
            for i, nrow in ((0, 3), (1, 2)):
                stg = scr.tile([128, 512], F32, tag="stage")
                nc.scalar.copy(out=stg[0:97], in_=pw[i][0:97])
                nc.sync.dma_start(
                    out=wsq_scr[3 * i:3 * i + nrow],
                    in_=stg.rearrange("(a b) f -> a b f", b=32)[0:nrow, 0])
            nc.sync.dma_start(out=wsq, in_=wsq_scr[:].rearrange(
                "a (b p) -> p (a b)", p=128))

            nc.sync.dma_start(out=egsb, in_=eg_d[:])
            nc.sync.dma_start(out=ersb, in_=er_d[:])
            nc.sync.dma_start(out=wlsb, in_=wl_d[:])
            nc.sync.dma_start(out=selsb, in_=sel_d[:])

            # winv first: it gates the whole W prep pipeline
            rsqrt_dve(winv, wsq, WCH, scale=float(AM_SCALE))

            # W normalise (x30) into bf16, PE transpose, ACT copy casts to fp8
            w8n = big.tile([128, WCH, D], BF16)
            for c in range(WCH):
                nc.vector.tensor_scalar_mul(w8n[:, c], w8[:, c],
                                            winv[:, c:c + 1])
                pt = tp.tile([128, 2, 128], BF16, tag="tp")
                for kd in range(2):
                    nc.tensor.transpose(pt[:, kd],
                                        w8n[:, c, kd * 128:(kd + 1) * 128],
                                        identb)
                nc.scalar.copy(out=WT8[:, :, c * 128:(c + 1) * 128], in_=pt)

            # esq: et8 o et8 -> bf16, ones-matmul per 512-block
            nc.sync.dma_start(out=et8[:, :, 0:2048], in_=et8_d[:, :, 0:2048])
            nc.sync.dma_start(out=et8[:, :, 2048:4096],
                              in_=et8_d[:, :, 2048:4096])
            esqsrc = big.tile([128, 2, B], BF16)
            for kd in range(2):
                for q in range(8):
                    sli = slice(q * 512, (q + 1) * 512)
                    nc.vector.tensor_mul(esqsrc[:, kd, sli], et8[:, kd, sli],
                                         et8[:, kd, sli])
            pe = []
            for i in range(3):
                pet = pp.tile([128, 512], F32, tag=f"pe{i}")
                pe.append(pet)
            for j in range(8):
                dst = pe[j // 3][(j % 3) * 32:(j % 3) * 32 + 1, :]
                for kd in range(2):
                    nc.tensor.matmul(dst, lhsT=ones_bf,
                                     rhs=esqsrc[:, kd, j * 512:(j + 1) * 512],
                                     start=(kd == 0), stop=(kd == 1))
            for i, nrow in ((0, 3), (1, 3), (2, 2)):
                stg = scr.tile([128, 512], F32, tag="stage")
                nc.scalar.copy(out=stg[0:97], in_=pe[i][0:97])
                nc.sync.dma_start(
                    out=esq_scr[3 * i:3 * i + nrow],
                    in_=stg.rearrange("(a b) f -> a b f", b=32)[0:nrow, 0])
            nc.sync.dma_start(out=esq, in_=esq_scr[:].rearrange(
                "a (b p) -> p (a b)", p=128))

            rsqrt_dve(rinv, esq, RCH)
            nc.vector.tensor_scalar(out=rinvc1, in0=rinv, scalar1=float(C1),
                                    scalar2=None, op0=ALU.mult)


        # ---------------- main loop ----------------------------------------
        asums = big.tile([128, RCH], F32)
        bsums = big.tile([128, RCH], F32)

        # two persistent code buffers; zero the 60 pad columns once
        code_tiles = []
        for ci in range(2):
            ct = big.tile([128, 1024], I16, tag=f"codes{ci}")
            nc.vector.memset(ct[:, 964:1024], 0)
            code_tiles.append(ct)

        DVE_COLS = 512   # DVE drains B1 fully; Pool drains B2

        with tc.tile_pool(name="pA", bufs=2, space="PSUM") as pA, \
             tc.tile_pool(name="pB", bufs=1, space="PSUM") as pB:
            def emit_pa(r):
                lhs = et8[:, :, r * 128:(r + 1) * 128]
                pa = pA.tile([128, CA], F32, tag="mma")
                for tb in range(3):
                    nc.tensor.matmul(pa[:, tb * 512:(tb + 1) * 512],
                                     lhsT=lhs,
                                     rhs=WT8[:, :, tb * 512:(tb + 1) * 512],
                                     start=True, stop=True, perf_mode=DR)
                return pa

        # pa_{r+1} is emitted before pb_r so PE never stalls the ACT stream
            pa_tiles = {0: emit_pa(0)}
            for r in range(RCH):
                if r + 1 < RCH:
                    pa_tiles[r + 1] = emit_pa(r + 1)
                pa = pa_tiles.pop(r)
                lhs = et8[:, :, r * 128:(r + 1) * 128]
                ct = code_tiles[r % 2]
                pb = pB.tile([128, 1024], F32, tag="mmb")
                for h in range(NB):
                    nc.tensor.matmul(
                        pb[:, h * 512:(h + 1) * 512], lhsT=lhs,
                        rhs=WT8[:, :, CA + h * 512:CA + (h + 1) * 512],
                        start=True, stop=True, perf_mode=DR)
                nc.vector.tensor_scalar(
                    out=ct[:, 0:964], in0=pb[:, 0:964],
                    scalar1=rinvc1[:, r:r + 1], scalar2=float(C2),
                    op0=ALU.mult, op1=ALU.add)
                # ACT: exp + fused row-sum over the A share
                s1 = scr.tile([128, CA], BF16, tag="expscr")
                nc.scalar.activation(out=s1, in_=pa, func=AF.Exp,
                                     scale=rinv[:, r:r + 1],
                                     bias=negoff[:, 0:1],
                                     accum_out=asums[:, r:r + 1])
                # Pool does tree level 1, DVE level 2 + final reduce
                cb = ct.bitcast(BF16)
                t1 = scr.tile([128, 512], BF16, tag="tree1")
                nc.gpsimd.tensor_tensor(out=t1, in0=cb[:, 0:512],
                                        in1=cb[:, 512:1024], op=ALU.add)
                t2 = scr.tile([128, 256], BF16, tag="tree2")
                nc.vector.tensor_tensor(out=t2, in0=t1[:, 0:256],
                                        in1=t1[:, 256:512], op=ALU.add)
                nc.vector.tensor_reduce(out=bsums[:, r:r + 1], in_=t2,
                                        axis=AXL.X, op=ALU.add)

        sums = big.tile([128, RCH], F32)
        nc.vector.tensor_add(sums, asums, bsums)
        nc.sync.dma_start(out=out_s[:], in_=sums)

        # ---------------- tail: intra + label pieces ------------------------
        with tc.tile_pool(name="tail", bufs=1, space="PSUM") as tpp:
            # intra: normalise eg rows, selection-matmul, ||sum_g||^2
            egsq = big.tile([128, 4], F32)
            egs = scr.tile([128, 4, D], BF16, tag="egs")
            nc.gpsimd.tensor_mul(egs, egsb, egsb)
            nc.vector.tensor_reduce(out=egsq, in_=egs, axis=AXL.X, op=ALU.add)
            eginv = big.tile([128, 4], F32)
            rsqrt_dve(eginv, egsq, 4)
            for j in range(4):
                nc.vector.tensor_scalar_mul(egsb[:, j], egsb[:, j],
                                            eginv[:, j:j + 1])
            sg = tpp.tile([GPC, D], F32, tag="sg")
            for j in range(4):
                nc.tensor.matmul(sg, lhsT=selsb, rhs=egsb[:, j],
                                 start=(j == 0), stop=(j == 3))
            sgsb = scr.tile([GPC, D], BF16, tag="sgsb")
            nc.vector.tensor_copy(sgsb, sg)
            sgsq = scr.tile([GPC, D], BF16, tag="sgsq")
            nc.vector.tensor_mul(sgsq, sgsb, sgsb)
            ssq = big.tile([GPC, 1], F32)
            nc.vector.tensor_reduce(out=ssq, in_=sgsq, axis=AXL.X, op=ALU.add)
            npairs = NSAMP * (NSAMP - 1) / 2.0
            iv = big.tile([GPC, 1], F32)
            nc.vector.tensor_scalar(
                out=iv, in0=ssq,
                scalar1=-1.0 / (2.0 * npairs),
                scalar2=(1.0 - INTRA_MARGIN) + NSAMP / (2.0 * npairs),
                op0=ALU.mult, op1=ALU.add)
            nc.vector.tensor_scalar_max(iv, iv, 0.0)
            nc.sync.dma_start(out=out_iv[:], in_=iv)

            # label: tt = <er,wl>, ersq, wlsq (host does cos + sqrt)
            lcpack = big.tile([128, 12], F32)
            for (o, a, b) in ((0, ersb, wlsb), (4, ersb, ersb),
                              (8, wlsb, wlsb)):
                m = scr.tile([128, 4, D], BF16, tag="lcm")
                nc.gpsimd.tensor_mul(m, a, b)
                nc.vector.tensor_reduce(out=lcpack[:, o:o + 4], in_=m,
                                        axis=AXL.X, op=ALU.add)
            nc.sync.dma_start(out=out_lc[:], in_=lcpack)

    nc.finalize()
    return nc


def kernel(embeddings, labels, weight):
    e = np.ascontiguousarray(embeddings, dtype=np.float32)
    lab = np.asarray(labels).astype(np.int64)
    w = np.ascontiguousarray(weight, dtype=np.float32)
    assert e.shape == (B, D) and w.shape == (C, D) and lab.shape == (B,)

    members = np.argsort(lab, kind="stable").reshape(G, NSAMP)
    sel = np.tile(np.eye(GPC, dtype=np.float32), (2, 1)).astype(
        ml_dtypes.bfloat16)

    # host-side casts / layout moves (no arithmetic)
    et8_full = np.ascontiguousarray(e.T).astype(ml_dtypes.float8_e4m3)
    et8 = np.ascontiguousarray(et8_full.reshape(2, 128, B).transpose(1, 0, 2))

    in_maps = []
    for k in range(NCORES):
        wsh = np.zeros((CSH, D), np.float32)
        wsh[:CREAL] = w[k * CREAL:(k + 1) * CREAL]
        w8f = wsh.astype(ml_dtypes.float8_e4m3)
        w8 = np.ascontiguousarray(
            w8f.reshape(WCH, 128, D).transpose(1, 0, 2))
        wt8f = np.ascontiguousarray(w8f.T)              # [D, CSH]
        wt8 = np.ascontiguousarray(
            wt8f.reshape(2, 128, CSH).transpose(1, 0, 2))
        rows = slice(k * RPC, (k + 1) * RPC)
        er = e[rows].astype(ml_dtypes.bfloat16)
        wl = np.ascontiguousarray(w[lab[rows]]).astype(ml_dtypes.bfloat16)
        gm = members[k * GPC:(k + 1) * GPC]
        eg_idx = gm.T.reshape(-1)
        eg = np.ascontiguousarray(e[eg_idx]).astype(ml_dtypes.bfloat16)
        in_maps.append({
            "et8": et8, "w8": w8, "wt8": wt8,
            "er": np.ascontiguousarray(er.reshape(4, 128, D).transpose(1, 0, 2)),
            "wl": np.ascontiguousarray(wl.reshape(4, 128, D).transpose(1, 0, 2)),
            "eg": np.ascontiguousarray(eg.reshape(4, 128, D).transpose(1, 0, 2)),
            "sel": sel,
        })

    nc = build_program()
    res = run_bass_kernel_spmd(nc, in_maps, core_ids=list(range(NCORES)))
    global _last_results
    _last_results = res

    # ---------------- host combine (O(B), float64) -----------------------
    S = np.zeros(B, np.float64)
    for k in range(NCORES):
        S += res.results[k]["out_s"].T.reshape(B).astype(np.float64)
    cls = []
    for k in range(NCORES):
        pk = res.results[k]["out_lc"].astype(np.float64)
        # [128, 12] -> rows (c p): col j covers rows j*128..(j+1)*128
        tt = pk[:, 0:4].T.reshape(RPC)
        ersq = pk[:, 4:8].T.reshape(RPC)
        wlsq = pk[:, 8:12].T.reshape(RPC)
        cls.append(tt / np.sqrt(ersq * wlsq))
    cl = np.concatenate(cls)

    s, m = float(AM_SCALE), float(AM_MARGIN)
    S_adj = S - np.exp(s * cl - OFF) + np.exp(s * (cl - m) - OFF)
    am_i = (np.log(S_adj) + OFF) - s * (cl - m)
    am = am_i.mean()

    ivals = np.concatenate(
        [res.results[k]["out_iv"][:, 0] for k in range(NCORES)]
    ).astype(np.float64)
    intra = ivals.sum() / G
    total = am + LAMBDA_INTRA * intra
    return (np.float32(total), np.float32(am), np.float32(intra))
